# revision 5
# baseline (speedup 1.0000x reference)
"""Trainium2 Bass kernel for the AMM sparse-attention module.

Math (reference):
    P_src = concat([0.01*feat_src, lmk_src], ch).reshape(4096, 392)   (raw reshape)
    P_ref = concat([0.01*feat_ref, lmk_ref], ch).reshape(392, 4096)
    A     = softmax(P_src @ P_ref, axis=0) * M           (M = mask_ref==mask_src, cols)
    beta  = feat_ref . conv1_w ;  gama = feat_ref . conv2_w     (per ref pixel)
    out   = (A @ gama) * feat_src + (A @ beta)

Because softmax is over dim 0 (rows) and the A@vec contractions are over
columns, sharding the 4096 columns of A across the 8 cores makes the softmax
entirely core-local.  Each core computes S^T = (P_ref_shard)^T-contracted
against P_src via TensorE float32r matmuls (full PE rate, ~fp32 precision),
exponentiates on ScalarE with a fused free-axis accumulation (the softmax
denominator comes for free), forms per-column scalars c = M*beta/d, and runs a
second PE pass partial = c^T @ E^T giving this core's contribution to
(A@beta, A@gama) for all 4096 src pixels.  One 32 KB ReduceScatter both sums
the 8 partials and hands core k exactly its 512-pixel output window, so the
SPMD program needs no rank-dependent addressing.  The final
gama_hat*feat_src+beta_hat is a single ScalarE affine pass in pixel-major
layout (per-partition scale/bias).
"""

import sys

for _p in ("/opt/trn_rl_repo",):
    if _p not in sys.path:
        sys.path.insert(0, _p)

import numpy as np

import concourse.bass as bass
import concourse.bacc as bacc
import concourse.tile as tile
import concourse.mybir as mybir
from concourse.bass_utils import run_bass_kernel_spmd

N_CORES = 8
H = W = 64
HW = H * W                      # 4096
C_FEAT = 256
C_LMK = 136
CK = C_FEAT + C_LMK             # 392 contraction dim
SHARD = HW // N_CORES           # 512 columns of A per core
VISUAL_WEIGHT = 0.01

# raw-reshape boundary: element (i, k) of P_src is visual iff i*392+k < 256*4096
BND_COL = (C_FEAT * HW) // CK          # 2674 : columns of P_srcT fully visual below
BND_K = C_FEAT * HW - BND_COL * CK     # 368  : at i==2674, k<368 is visual

F32 = mybir.dt.float32
F32R = mybir.dt.float32r
I32 = mybir.dt.int32
AF = mybir.ActivationFunctionType
ALU = mybir.AluOpType

K_TILES = ((0, 128), (128, 128), (256, 128), (384, CK - 384))  # (start, size)
N_JT = 4            # 128-wide tiles of this core's 512 columns
N_CHUNK = 8         # 512-wide chunks of the 4096 src pixels

_NC_CACHE = []


def _build():
    nc = bacc.Bacc("TRN2", target_bir_lowering=False, debug=False,
                   num_devices=N_CORES)

    psrct_e = nc.dram_tensor("psrct", [CK, HW], F32, kind="ExternalInput")
    pref_e = nc.dram_tensor("pref", [CK, SHARD], F32, kind="ExternalInput")
    fsrct_e = nc.dram_tensor("fsrct", [SHARD, C_FEAT], F32, kind="ExternalInput")
    wmat_e = nc.dram_tensor("wmat", [128, 4], F32, kind="ExternalInput")
    bvec_e = nc.dram_tensor("bvec", [128, 2], F32, kind="ExternalInput")
    msrc_e = nc.dram_tensor("msrc", [128, N_JT], I32, kind="ExternalInput")
    mref_e = nc.dram_tensor("mref", [128, N_JT], I32, kind="ExternalInput")
    out_e = nc.dram_tensor("out", [SHARD, C_FEAT], F32, kind="ExternalOutput")

    # collective bounce buffers
    rs_in = nc.dram_tensor("rs_in", [N_CORES, 2, SHARD], F32)
    rs_out = nc.dram_tensor("rs_out", [2, SHARD], F32)

    # per-partition scale for the one split column of P_srcT inside k-tile 2
    _bsc = np.where(np.arange(128) + 256 < BND_K, VISUAL_WEIGHT, 1.0)
    bsc_e = nc.inline_tensor(_bsc.reshape(128, 1).astype(np.float32), name="bsc")

    with tile.TileContext(nc) as tc:
        with (
            tc.tile_pool(name="big", bufs=1) as big,
            tc.tile_pool(name="stage", bufs=6) as stage,
            tc.tile_pool(name="small", bufs=1) as small,
            tc.tile_pool(name="gemm_ps", bufs=4, space="PSUM") as gemm_ps,
            tc.tile_pool(name="bh_ps", bufs=2, space="PSUM") as bh_ps,
            tc.tile_pool(name="beta_ps", bufs=2, space="PSUM") as beta_ps,
        ):
            # persistent SBUF tensors
            psrcr = big.tile([128, 4 * HW], F32R, tag="psrcr")   # rhs, k-tile t at cols [t*4096,)
            e_sb = big.tile([128, N_JT * HW], F32R, tag="esb")   # exp(S^T), j-tile at [j*4096,)
            prefu = big.tile([128, 4 * 512], F32, tag="prefu")   # unscaled P_ref shard
            prefr = big.tile([128, 4 * 512], F32R, tag="prefr")  # scaled lhsT
            wmat_sb = small.tile([128, 4], F32, tag="wmat")
            bvec_sb = small.tile([128, 2], F32, tag="bvec")
            msrc_sb = small.tile([128, N_JT], I32, tag="msrc")
            mref_sb = small.tile([128, N_JT], I32, tag="mref")
            mask_sb = small.tile([128, N_JT], F32, tag="mask")
            dpart = small.tile([128, N_JT * N_CHUNK], F32, tag="dpart")
            dsum = small.tile([128, N_JT], F32, tag="dsum")
            drec = small.tile([128, N_JT], F32, tag="drec")
            betab = small.tile([128, 2 * N_JT], F32, tag="betab")
            c_r = small.tile([128, 2 * N_JT], F32R, tag="cr")
            part_sb = small.tile([2, HW], F32, tag="part")
            bhb = small.tile([128, N_JT], F32, tag="bhb")
            ghb = small.tile([128, N_JT], F32, tag="ghb")
            fst_sb = big.tile([128, N_JT * C_FEAT], F32, tag="fst")
            outt_sb = big.tile([128, N_JT * C_FEAT], F32, tag="outt")
            bsc_sb = small.tile([128, 1], F32, tag="bsc")
            nc.sync.dma_start(bsc_sb[:], bsc_e[:])

            # small inputs
            nc.sync.dma_start(wmat_sb[:], wmat_e[:])
            nc.sync.dma_start(bvec_sb[:], bvec_e[:])
            nc.sync.dma_start(msrc_sb[:], msrc_e[:])
            nc.sync.dma_start(mref_sb[:], mref_e[:])
            nc.vector.tensor_tensor(out=mask_sb[:], in0=mref_sb[:], in1=msrc_sb[:],
                                    op=ALU.is_equal)
            for t in range(N_JT):
                nc.sync.dma_start(fst_sb[:, t * C_FEAT:(t + 1) * C_FEAT],
                                  fsrct_e[t * 128:(t + 1) * 128, :])

            # P_ref shard: load unscaled, then make the float32r scaled copy
            for t, (ks, kn) in enumerate(K_TILES):
                nc.sync.dma_start(prefu[:kn, t * 512:t * 512 + 512],
                                  pref_e[ks:ks + kn, :])
                sl_u = prefu[:kn, t * 512:t * 512 + 512]
                sl_r = prefr[:kn, t * 512:t * 512 + 512]
                if ks < C_FEAT:  # visual channels -> scale
                    nc.vector.tensor_scalar_mul(sl_r, sl_u, VISUAL_WEIGHT)
                else:
                    nc.vector.tensor_copy(sl_r, sl_u)

            # beta/gama for this core's columns: betab[:, 2j:2j+2]
            for j in range(N_JT):
                bps = beta_ps.tile([128, 2], F32, tag="beta")
                for t in (0, 1):
                    nc.tensor.matmul(
                        bps[:],
                        prefu[:, t * 512 + j * 128:t * 512 + (j + 1) * 128],
                        wmat_sb[:, 2 * t:2 * t + 2],
                        start=(t == 0), stop=(t == 1),
                    )
                nc.vector.tensor_tensor(out=betab[:, 2 * j:2 * j + 2], in0=bps[:],
                                        in1=bvec_sb[:], op=ALU.add)

            # P_srcT: DMA chunk stages, scale visual region, round to float32r
            for t, (ks, kn) in enumerate(K_TILES):
                for c in range(N_CHUNK):
                    st = stage.tile([128, 512], F32, tag="stage")
                    nc.sync.dma_start(st[:kn, :], psrct_e[ks:ks + kn,
                                                          c * 512:(c + 1) * 512])
                    dst = psrcr[:kn, t * HW + c * 512: t * HW + (c + 1) * 512]
                    c0 = c * 512
                    s_end = min(BND_COL, c0 + 512) - c0  # local cols [0, s_end) visual
                    if s_end >= 512:
                        nc.vector.tensor_scalar_mul(dst, st[:kn, :], VISUAL_WEIGHT)
                        continue
                    if s_end > 0:
                        nc.vector.tensor_scalar_mul(dst[:, :s_end], st[:kn, :s_end],
                                                    VISUAL_WEIGHT)
                    bl = BND_COL - c0  # local index of the split column, if inside
                    if 0 <= bl < 512:
                        col_d = dst[:, bl:bl + 1]
                        col_s = st[:kn, bl:bl + 1]
                        if ks + kn <= BND_K:      # fully visual column
                            nc.vector.tensor_scalar_mul(col_d, col_s, VISUAL_WEIGHT)
                        elif ks >= BND_K:         # fully landmark column
                            nc.vector.tensor_copy(col_d, col_s)
                        else:                     # split inside this k-tile
                            nc.vector.tensor_scalar(
                                out=col_d, in0=col_s, scalar1=bsc_sb[:kn, :],
                                scalar2=None, op0=ALU.mult)
                        rest = bl + 1
                    else:
                        rest = max(s_end, 0)
                    if rest < 512:
                        nc.vector.tensor_copy(dst[:, rest:], st[:kn, rest:])

            # main GEMM: S^T[j-tile] chunks -> exp -> E, denominators via accum
            for j in range(N_JT):
                for c in range(N_CHUNK):
                    ps = gemm_ps.tile([128, 512], F32, tag="gps")
                    for t, (ks, kn) in enumerate(K_TILES):
                        nc.tensor.matmul(
                            ps[:],
                            prefr[:kn, t * 512 + j * 128:t * 512 + (j + 1) * 128],
                            psrcr[:kn, t * HW + c * 512:t * HW + (c + 1) * 512],
                            start=(t == 0), stop=(t == 3),
                        )
                    nc.scalar.activation(
                        e_sb[:, j * HW + c * 512:j * HW + (c + 1) * 512],
                        ps[:], AF.Exp, bias=0.0, scale=1.0,
                        accum_out=dpart[:, j * N_CHUNK + c:j * N_CHUNK + c + 1],
                    )
                # d_j, 1/d_j, c_j = (beta+b) * M / d   (written rounded to f32r)
                nc.vector.tensor_reduce(
                    dsum[:, j:j + 1], dpart[:, j * N_CHUNK:(j + 1) * N_CHUNK],
                    axis=mybir.AxisListType.X, op=ALU.add)
                nc.vector.reciprocal(drec[:, j:j + 1], dsum[:, j:j + 1])
                nc.vector.tensor_scalar(
                    out=c_r[:, 2 * j:2 * j + 2], in0=betab[:, 2 * j:2 * j + 2],
                    scalar1=mask_sb[:, j:j + 1], scalar2=drec[:, j:j + 1],
                    op0=ALU.mult, op1=ALU.mult)

            # second pass: partial[m, i] = sum_j c[j, m] * E^T[j, i]
            for c in range(N_CHUNK):
                bps = bh_ps.tile([2, 512], F32, tag="bh")
                for j in range(N_JT):
                    nc.tensor.matmul(
                        bps[:],
                        c_r[:, 2 * j:2 * j + 2],
                        e_sb[:, j * HW + c * 512:j * HW + (c + 1) * 512],
                        start=(j == 0), stop=(j == N_JT - 1),
                    )
                nc.vector.tensor_copy(part_sb[:, c * 512:(c + 1) * 512], bps[:])

            # combine partials across cores; core k receives its pixel window
            nc.sync.dma_start(rs_in[:, 0, :], part_sb[0:1, :])
            nc.sync.dma_start(rs_in[:, 1, :], part_sb[1:2, :])
            nc.gpsimd.collective_compute(
                "ReduceScatter", ALU.add,
                ins=[rs_in.ap().opt()],
                outs=[rs_out.ap().opt()],
                replica_groups=[list(range(N_CORES))],
            )
            rsv = rs_out.ap().rearrange("m (t p) -> m p t", p=128)
            nc.sync.dma_start(bhb[:], rsv[0])
            nc.sync.dma_start(ghb[:], rsv[1])

            # out^T[p, ch] = gama_hat[p] * feat_srcT[p, ch] + beta_hat[p]
            for t in range(N_JT):
                nc.scalar.activation(
                    outt_sb[:, t * C_FEAT:(t + 1) * C_FEAT],
                    fst_sb[:, t * C_FEAT:(t + 1) * C_FEAT],
                    AF.Identity,
                    bias=bhb[:, t:t + 1], scale=ghb[:, t:t + 1],
                )
                nc.sync.dma_start(out_e[t * 128:(t + 1) * 128, :],
                                  outt_sb[:, t * C_FEAT:(t + 1) * C_FEAT])

    nc.compile()
    return nc


def _get_nc():
    if not _NC_CACHE:
        _NC_CACHE.append(_build())
    return _NC_CACHE[0]


def _prep_in_maps(feat_src, feat_ref, landmarks_src, landmarks_ref,
                  mask_src, mask_ref, conv1_w, conv1_b, conv2_w, conv2_b):
    fs = np.asarray(feat_src, np.float32).reshape(C_FEAT, HW)
    fr = np.asarray(feat_ref, np.float32).reshape(C_FEAT, HW)
    ls = np.asarray(landmarks_src, np.float32).reshape(C_LMK, HW)
    lr = np.asarray(landmarks_ref, np.float32).reshape(C_LMK, HW)
    ms = np.asarray(mask_src, np.int32).reshape(HW)
    mr = np.asarray(mask_ref, np.int32).reshape(HW)

    src_cat = np.concatenate([fs, ls], axis=0)          # unscaled; device scales
    ref_cat = np.concatenate([fr, lr], axis=0)
    # P_srcT[k, i] = src_flat[i*392 + k]  (the raw-reshape de-interleave)
    psrct = np.ascontiguousarray(src_cat.reshape(-1).reshape(HW, CK).T)

    w1 = np.asarray(conv1_w, np.float32)[0, :, 0, 0]
    w2 = np.asarray(conv2_w, np.float32)[0, :, 0, 0]
    wmat = np.stack([w1, w2], axis=1)                   # (256, 2)
    wmat_t = np.ascontiguousarray(
        wmat.reshape(2, 128, 2).transpose(1, 0, 2).reshape(128, 4))
    bvec = np.broadcast_to(
        np.array([np.asarray(conv1_b, np.float32).reshape(-1)[0],
                  np.asarray(conv2_b, np.float32).reshape(-1)[0]], np.float32),
        (128, 2)).copy()

    in_maps = []
    for k in range(N_CORES):
        J = slice(k * SHARD, (k + 1) * SHARD)
        in_maps.append(dict(
            psrct=psrct,
            pref=np.ascontiguousarray(ref_cat[:, J]),
            fsrct=np.ascontiguousarray(fs[:, J].T),
            wmat=wmat_t,
            bvec=bvec,
            msrc=np.ascontiguousarray(ms[J].reshape(N_JT, 128).T),
            mref=np.ascontiguousarray(mr[J].reshape(N_JT, 128).T),
        ))
    return in_maps


def _assemble(results):
    outt = np.concatenate([results[k]["out"] for k in range(N_CORES)], axis=0)
    return np.ascontiguousarray(outt.T).reshape(1, C_FEAT, H, W)


def run(trace=False, **inputs):
    nc = _get_nc()
    in_maps = _prep_in_maps(**inputs)
    res = run_bass_kernel_spmd(nc, in_maps, core_ids=list(range(N_CORES)),
                               trace=trace)
    return _assemble(res.results), res


def kernel(**inputs) -> np.ndarray:
    out, _ = run(trace=False, **inputs)
    return out
